# revision 6
# baseline (speedup 1.0000x reference)
"""Block-sparse linear kernel for Trainium2 (8 NeuronCores, data-parallel).

Computes out = 2 * (x @ (weight*mask).T) + bias for
x: (8, 2048, 4096) f32, weight: (4096, 4096) f32, bias: (4096,) f32,
block_mask: (128, 128) bool over 32x32 blocks.

Strategy: shard x on batch across the 8 cores (weight/bias replicated).
The mask and the *2 scale are folded into the weight on the host, so each
core runs a dense M=2048, K=4096, N=4096 GEMM in bf16 with fp32 PSUM
accumulation. Both operands stream: x in s-slabs of 512 rows, weight.T in
o-chunks of 512 (reloaded per slab -- HBM bandwidth has slack, and the
small working set lets compute start ~4 MiB into the transfer instead of
waiting for a full 16 MiB x residency). Transfers are batched into ~1 MiB
dma_starts to keep the Sync queue shallow; output stores issue from the
Scalar engine and bias loads from GpSimd so they never queue ahead of
weight loads. Bias is added during PSUM->SBUF eviction on the vector
engine.
"""
import os

import numpy as np

# Problem constants (hardcoded per the harness contract).
B, S, IN, OUT = 8, 2048, 4096, 4096
BLOCK = 32
P = 128                    # partitions / contraction tile
IT = IN // P               # 32 i-tiles
OC = 512                   # o-chunk width (matmul free dim)
NOC = OUT // OC            # 8 o-chunks
SLAB = 512                 # s rows per slab
NSL = S // SLAB            # 4 slabs
STS = SLAB // P            # 4 s-tiles per slab
QI = IT // 4               # i-tiles per DMA quarter

LAST_EXEC_NS = None


def _build_program():
    import concourse.bacc as bacc
    import concourse.tile as tile
    from concourse import mybir

    bf16 = mybir.dt.bfloat16
    f32 = mybir.dt.float32

    nc = bacc.Bacc("TRN2", debug=False, num_devices=B)
    x_d = nc.dram_tensor("xt", (NSL, P, IT, SLAB), bf16, kind="ExternalInput")
    w_d = nc.dram_tensor("wt", (NOC, P, IT, OC), bf16, kind="ExternalInput")
    b_d = nc.dram_tensor("bias", (NOC, P, OC), f32, kind="ExternalInput")
    o_d = nc.dram_tensor("out", (S, OUT), f32, kind="ExternalOutput")

    with tile.TileContext(nc) as tc:
        with (
            tc.tile_pool(name="xpool", bufs=2) as xp,
            tc.tile_pool(name="wpool", bufs=3) as wp,
            tc.tile_pool(name="bpool", bufs=2) as bp,
            tc.tile_pool(name="opool", bufs=4) as op,
            tc.tile_pool(name="psum", bufs=4, space="PSUM") as pp,
        ):
            def load_w(oc):
                wc = wp.tile([P, IT, OC], bf16, tag="w", name="wc")
                for q in range(4):
                    nc.sync.dma_start(
                        out=wc[:, q * QI:(q + 1) * QI, :],
                        in_=w_d[oc, :, q * QI:(q + 1) * QI, :],
                    )
                return wc

            def load_x(sl):
                xs = xp.tile([P, IT, SLAB], bf16, tag="x", name="xs")
                for q in range(4):
                    nc.sync.dma_start(
                        out=xs[:, q * QI:(q + 1) * QI, :],
                        in_=x_d[sl, :, q * QI:(q + 1) * QI, :],
                    )
                return xs

            # PE warm-up: ~200 junk matmuls (no DMA deps, scheduled first)
            # keep the tensor engine busy through the HAM activity window
            # while the first real tiles are still in flight, so the real
            # matmuls start at the full 2.4 GHz clock.
            wj = xp.tile([P, P], bf16, tag="warm", name="wj")
            nc.vector.memset(wj[:], 0.0)
            psj = pp.tile([P, 64], f32, tag="psj", name="psj")
            for _ in range(200):
                nc.tensor.matmul(psj[:], wj[:], wj[:, :64], start=True, stop=True)

            for sl in range(NSL):
                if sl == 0:
                    # Interleave the first w chunk with the x slab in eighth
                    # chunks so the first accumulation can start ~1 MiB into
                    # the load.
                    wc0 = wp.tile([P, IT, OC], bf16, tag="w", name="wc")
                    xs = xp.tile([P, IT, SLAB], bf16, tag="x", name="xs")
                    E = IT // 8
                    for q in range(8):
                        nc.sync.dma_start(
                            out=wc0[:, q * E:(q + 1) * E, :],
                            in_=w_d[0, :, q * E:(q + 1) * E, :],
                        )
                        nc.sync.dma_start(
                            out=xs[:, q * E:(q + 1) * E, :],
                            in_=x_d[0, :, q * E:(q + 1) * E, :],
                        )
                else:
                    xs = load_x(sl)
                for oc in range(NOC):
                    wc = wc0 if sl == 0 and oc == 0 else load_w(oc)
                    bt = bp.tile([P, OC], f32, tag="b", name="bt")
                    nc.gpsimd.dma_start(out=bt[:], in_=b_d[oc])
                    for st in range(STS):
                        ps = pp.tile([P, OC], f32, tag="ps", name="ps")
                        for it in range(IT):
                            nc.tensor.matmul(
                                ps[:],
                                xs[:, it, st * P:(st + 1) * P],
                                wc[:, it, :],
                                start=(it == 0),
                                stop=(it == IT - 1),
                            )
                        ot = op.tile([P, OC], f32, tag="o", name="ot")
                        nc.vector.tensor_add(out=ot[:], in0=ps[:], in1=bt[:])
                        nc.scalar.dma_start(
                            out=o_d[
                                sl * SLAB + st * P:sl * SLAB + (st + 1) * P,
                                oc * OC:(oc + 1) * OC,
                            ],
                            in_=ot[:],
                        )
    nc.compile()
    return nc


def _install_axon_ntff_hook(so_path="/opt/axon/libaxon_pjrt.so"):
    """Make run_bass_kernel_spmd(trace=True) work when the image's antenv
    lacks axon_hooks: drive NTFF profiling via ctypes on libaxon_pjrt.so."""
    import contextlib
    import ctypes
    import sys
    import types

    lib = ctypes.CDLL(so_path)
    if not hasattr(lib, "axon_start_nrt_profile"):
        return
    lib.axon_start_nrt_profile.argtypes = [
        ctypes.POINTER(ctypes.c_int64),
        ctypes.c_size_t,
    ]
    lib.axon_start_nrt_profile.restype = ctypes.c_int64
    lib.axon_stop_nrt_profile.argtypes = [ctypes.c_char_p]
    lib.axon_stop_nrt_profile.restype = ctypes.c_int64

    @contextlib.contextmanager
    def _hook(output_dir, device_ids):
        import jax

        jax.devices()
        if device_ids:
            ids = (ctypes.c_int64 * len(device_ids))(*device_ids)
            rc = lib.axon_start_nrt_profile(ids, len(device_ids))
        else:
            rc = lib.axon_start_nrt_profile(None, 0)
        if rc != 0:
            raise RuntimeError(f"axon_start_nrt_profile rc={rc}")
        try:
            yield
        finally:
            n = lib.axon_stop_nrt_profile(str(output_dir).encode())
            print(f"ntff profile: {n} file(s) -> {output_dir}", file=sys.stderr)

    mod = types.ModuleType("antenv.axon_hooks")
    mod.get_axon_ntff_profile_hook = lambda: _hook
    mod.set_axon_ntff_profile_hook = lambda h: None
    sys.modules["antenv.axon_hooks"] = mod

    import concourse.bass_utils as bu

    bu.upload_artifacts = lambda tmpdir: f"file://{tmpdir}"


def kernel(x, weight, bias, block_mask):
    global LAST_EXEC_NS
    from concourse.bass_utils import run_bass_kernel_spmd
    from concourse import mybir

    bf16 = mybir.dt.np(mybir.dt.bfloat16)

    # Host-side prep: fold mask and the x2 into the weight, pre-transpose.
    mask = np.repeat(np.repeat(np.asarray(block_mask), BLOCK, 0), BLOCK, 1)
    w_eff = (2.0 * np.asarray(weight, np.float32)) * mask
    wt = np.ascontiguousarray(w_eff.T)                       # [IN, OUT]
    # [NOC, P, IT, OC]: per (oc, partition) a contiguous IT*OC*2-byte run.
    w_dev = np.ascontiguousarray(
        wt.reshape(IT, P, NOC, OC).transpose(2, 1, 0, 3)
    ).astype(bf16)
    b_dev = np.ascontiguousarray(
        np.broadcast_to(
            np.asarray(bias, np.float32).reshape(NOC, 1, OC), (NOC, P, OC)
        )
    )

    xs = np.asarray(x, np.float32)
    in_maps = []
    for b in range(B):
        # [NSL, P, IT, SLAB]: per (slab, partition) contiguous IT*SLAB*2 run.
        x_dev = np.ascontiguousarray(
            xs[b].T.reshape(IT, P, NSL, SLAB).transpose(2, 1, 0, 3)
        ).astype(bf16)
        in_maps.append({"xt": x_dev, "wt": w_dev, "bias": b_dev})

    nc = _build_program()
    trace = bool(int(os.environ.get("BSL_TRACE", "0")))
    if trace:
        _install_axon_ntff_hook()
    res = run_bass_kernel_spmd(
        nc, in_maps, list(range(B)), trace=trace,
    )
    LAST_EXEC_NS = res.exec_time_ns
    return np.stack([res.results[b]["out"] for b in range(B)]).astype(np.float32)


# revision 7
# speedup vs baseline: 1.0059x; 1.0059x over previous
"""Block-sparse linear kernel for Trainium2 (8 NeuronCores, data-parallel).

Computes out = 2 * (x @ (weight*mask).T) + bias for
x: (8, 2048, 4096) f32, weight: (4096, 4096) f32, bias: (4096,) f32,
block_mask: (128, 128) bool over 32x32 blocks.

Strategy: shard x on batch across the 8 cores (weight/bias replicated).
The mask and the *2 scale are folded into the weight on the host, so each
core runs a dense M=2048, K=4096, N=4096 GEMM in bf16 with fp32 PSUM
accumulation. Both operands stream: x in s-slabs of 512 rows, weight.T in
o-chunks of 512 (reloaded per slab -- HBM bandwidth has slack, and the
small working set lets compute start ~4 MiB into the transfer instead of
waiting for a full 16 MiB x residency). Transfers are batched into ~1 MiB
dma_starts to keep the Sync queue shallow; output stores issue from the
Scalar engine and bias loads from GpSimd so they never queue ahead of
weight loads. Bias is added during PSUM->SBUF eviction on the vector
engine.
"""
import os

import numpy as np

# Problem constants (hardcoded per the harness contract).
B, S, IN, OUT = 8, 2048, 4096, 4096
BLOCK = 32
P = 128                    # partitions / contraction tile
IT = IN // P               # 32 i-tiles
OC = 512                   # o-chunk width (matmul free dim)
NOC = OUT // OC            # 8 o-chunks
SLAB = 512                 # s rows per slab
NSL = S // SLAB            # 4 slabs
STS = SLAB // P            # 4 s-tiles per slab
QI = IT // 4               # i-tiles per DMA quarter

LAST_EXEC_NS = None


def _build_program():
    import concourse.bacc as bacc
    import concourse.tile as tile
    from concourse import mybir

    bf16 = mybir.dt.bfloat16
    f32 = mybir.dt.float32

    nc = bacc.Bacc("TRN2", debug=False, num_devices=B)
    x_d = nc.dram_tensor("xt", (NSL, P, IT, SLAB), bf16, kind="ExternalInput")
    w_d = nc.dram_tensor("wt", (NOC, P, IT, OC), bf16, kind="ExternalInput")
    b_d = nc.dram_tensor("bias", (NOC, P, OC), f32, kind="ExternalInput")
    o_d = nc.dram_tensor("out", (S, OUT), f32, kind="ExternalOutput")

    with tile.TileContext(nc) as tc:
        with (
            tc.tile_pool(name="xpool", bufs=2) as xp,
            tc.tile_pool(name="wpool", bufs=3) as wp,
            tc.tile_pool(name="bpool", bufs=2) as bp,
            tc.tile_pool(name="opool", bufs=4) as op,
            tc.tile_pool(name="psum", bufs=4, space="PSUM") as pp,
        ):
            def load_w(oc):
                wc = wp.tile([P, IT, OC], bf16, tag="w", name="wc")
                for q in range(4):
                    nc.sync.dma_start(
                        out=wc[:, q * QI:(q + 1) * QI, :],
                        in_=w_d[oc, :, q * QI:(q + 1) * QI, :],
                    )
                return wc

            def load_x(sl):
                xs = xp.tile([P, IT, SLAB], bf16, tag="x", name="xs")
                for q in range(4):
                    nc.sync.dma_start(
                        out=xs[:, q * QI:(q + 1) * QI, :],
                        in_=x_d[sl, :, q * QI:(q + 1) * QI, :],
                    )
                return xs

            # PE warm-up: ~200 junk matmuls (no DMA deps, scheduled first)
            # keep the tensor engine busy through the HAM activity window
            # while the first real tiles are still in flight, so the real
            # matmuls start at the full 2.4 GHz clock.
            wj = xp.tile([P, P], bf16, tag="warm", name="wj")
            nc.vector.memset(wj[:], 0.0)
            psj = pp.tile([P, 64], f32, tag="psj", name="psj")
            for _ in range(120):
                nc.tensor.matmul(psj[:], wj[:], wj[:, :64], start=True, stop=True)

            for sl in range(NSL):
                if sl == 0:
                    # Interleave the first w chunk with the x slab in eighth
                    # chunks so the first accumulation can start ~1 MiB into
                    # the load.
                    wc0 = wp.tile([P, IT, OC], bf16, tag="w", name="wc")
                    xs = xp.tile([P, IT, SLAB], bf16, tag="x", name="xs")
                    E = IT // 8
                    for q in range(8):
                        nc.sync.dma_start(
                            out=wc0[:, q * E:(q + 1) * E, :],
                            in_=w_d[0, :, q * E:(q + 1) * E, :],
                        )
                        nc.sync.dma_start(
                            out=xs[:, q * E:(q + 1) * E, :],
                            in_=x_d[0, :, q * E:(q + 1) * E, :],
                        )
                else:
                    xs = load_x(sl)
                for oc in range(NOC):
                    wc = wc0 if sl == 0 and oc == 0 else load_w(oc)
                    bt = bp.tile([P, OC], f32, tag="b", name="bt")
                    nc.gpsimd.dma_start(out=bt[:], in_=b_d[oc])
                    for st in range(STS):
                        ps = pp.tile([P, OC], f32, tag="ps", name="ps")
                        for it in range(IT):
                            nc.tensor.matmul(
                                ps[:],
                                xs[:, it, st * P:(st + 1) * P],
                                wc[:, it, :],
                                start=(it == 0),
                                stop=(it == IT - 1),
                            )
                        ot = op.tile([P, OC], f32, tag="o", name="ot")
                        nc.vector.tensor_add(out=ot[:], in0=ps[:], in1=bt[:])
                        nc.scalar.dma_start(
                            out=o_d[
                                sl * SLAB + st * P:sl * SLAB + (st + 1) * P,
                                oc * OC:(oc + 1) * OC,
                            ],
                            in_=ot[:],
                        )
    nc.compile()
    return nc


def _install_axon_ntff_hook(so_path="/opt/axon/libaxon_pjrt.so"):
    """Make run_bass_kernel_spmd(trace=True) work when the image's antenv
    lacks axon_hooks: drive NTFF profiling via ctypes on libaxon_pjrt.so."""
    import contextlib
    import ctypes
    import sys
    import types

    lib = ctypes.CDLL(so_path)
    if not hasattr(lib, "axon_start_nrt_profile"):
        return
    lib.axon_start_nrt_profile.argtypes = [
        ctypes.POINTER(ctypes.c_int64),
        ctypes.c_size_t,
    ]
    lib.axon_start_nrt_profile.restype = ctypes.c_int64
    lib.axon_stop_nrt_profile.argtypes = [ctypes.c_char_p]
    lib.axon_stop_nrt_profile.restype = ctypes.c_int64

    @contextlib.contextmanager
    def _hook(output_dir, device_ids):
        import jax

        jax.devices()
        if device_ids:
            ids = (ctypes.c_int64 * len(device_ids))(*device_ids)
            rc = lib.axon_start_nrt_profile(ids, len(device_ids))
        else:
            rc = lib.axon_start_nrt_profile(None, 0)
        if rc != 0:
            raise RuntimeError(f"axon_start_nrt_profile rc={rc}")
        try:
            yield
        finally:
            n = lib.axon_stop_nrt_profile(str(output_dir).encode())
            print(f"ntff profile: {n} file(s) -> {output_dir}", file=sys.stderr)

    mod = types.ModuleType("antenv.axon_hooks")
    mod.get_axon_ntff_profile_hook = lambda: _hook
    mod.set_axon_ntff_profile_hook = lambda h: None
    sys.modules["antenv.axon_hooks"] = mod

    import concourse.bass_utils as bu

    bu.upload_artifacts = lambda tmpdir: f"file://{tmpdir}"


def kernel(x, weight, bias, block_mask):
    global LAST_EXEC_NS
    from concourse.bass_utils import run_bass_kernel_spmd
    from concourse import mybir

    bf16 = mybir.dt.np(mybir.dt.bfloat16)

    # Host-side prep: fold mask and the x2 into the weight, pre-transpose.
    mask = np.repeat(np.repeat(np.asarray(block_mask), BLOCK, 0), BLOCK, 1)
    w_eff = (2.0 * np.asarray(weight, np.float32)) * mask
    wt = np.ascontiguousarray(w_eff.T)                       # [IN, OUT]
    # [NOC, P, IT, OC]: per (oc, partition) a contiguous IT*OC*2-byte run.
    w_dev = np.ascontiguousarray(
        wt.reshape(IT, P, NOC, OC).transpose(2, 1, 0, 3)
    ).astype(bf16)
    b_dev = np.ascontiguousarray(
        np.broadcast_to(
            np.asarray(bias, np.float32).reshape(NOC, 1, OC), (NOC, P, OC)
        )
    )

    xs = np.asarray(x, np.float32)
    in_maps = []
    for b in range(B):
        # [NSL, P, IT, SLAB]: per (slab, partition) contiguous IT*SLAB*2 run.
        x_dev = np.ascontiguousarray(
            xs[b].T.reshape(IT, P, NSL, SLAB).transpose(2, 1, 0, 3)
        ).astype(bf16)
        in_maps.append({"xt": x_dev, "wt": w_dev, "bias": b_dev})

    nc = _build_program()
    trace = bool(int(os.environ.get("BSL_TRACE", "0")))
    if trace:
        _install_axon_ntff_hook()
    res = run_bass_kernel_spmd(
        nc, in_maps, list(range(B)), trace=trace,
    )
    LAST_EXEC_NS = res.exec_time_ns
    return np.stack([res.results[b]["out"] for b in range(B)]).astype(np.float32)


# revision 8
# speedup vs baseline: 1.0060x; 1.0000x over previous
"""Block-sparse linear kernel for Trainium2 (8 NeuronCores, data-parallel).

Computes out = 2 * (x @ (weight*mask).T) + bias for
x: (8, 2048, 4096) f32, weight: (4096, 4096) f32, bias: (4096,) f32,
block_mask: (128, 128) bool over 32x32 blocks.

Strategy: shard x on batch across the 8 cores (weight/bias replicated).
The mask and the *2 scale are folded into the weight on the host, so each
core runs a dense M=2048, K=4096, N=4096 GEMM in bf16 with fp32 PSUM
accumulation. Both operands stream: x in s-slabs of 512 rows, weight.T in
o-chunks of 512 (reloaded per slab -- HBM bandwidth has slack, and the
small working set lets compute start ~4 MiB into the transfer instead of
waiting for a full 16 MiB x residency). Transfers are batched into ~1 MiB
dma_starts to keep the Sync queue shallow; output stores issue from the
Scalar engine and bias loads from GpSimd so they never queue ahead of
weight loads. Bias is added during PSUM->SBUF eviction on the vector
engine.
"""
import os

import numpy as np

# Problem constants (hardcoded per the harness contract).
B, S, IN, OUT = 8, 2048, 4096, 4096
BLOCK = 32
P = 128                    # partitions / contraction tile
IT = IN // P               # 32 i-tiles
OC = 512                   # o-chunk width (matmul free dim)
NOC = OUT // OC            # 8 o-chunks
SLAB = 512                 # s rows per slab
NSL = S // SLAB            # 4 slabs
STS = SLAB // P            # 4 s-tiles per slab
QI = IT // 4               # i-tiles per DMA quarter

LAST_EXEC_NS = None


def _build_program():
    import concourse.bacc as bacc
    import concourse.tile as tile
    from concourse import mybir

    bf16 = mybir.dt.bfloat16
    f32 = mybir.dt.float32

    nc = bacc.Bacc("TRN2", debug=False, num_devices=B)
    x_d = nc.dram_tensor("xt", (NSL, P, IT, SLAB), bf16, kind="ExternalInput")
    w_d = nc.dram_tensor("wt", (NOC, P, IT, OC), bf16, kind="ExternalInput")
    b_d = nc.dram_tensor("bias", (NOC, P, OC), f32, kind="ExternalInput")
    o_d = nc.dram_tensor("out", (S, OUT), f32, kind="ExternalOutput")

    with tile.TileContext(nc) as tc:
        with (
            tc.tile_pool(name="xpool", bufs=2) as xp,
            tc.tile_pool(name="wpool", bufs=3) as wp,
            tc.tile_pool(name="bpool", bufs=2) as bp,
            tc.tile_pool(name="opool", bufs=4) as op,
            tc.tile_pool(name="psum", bufs=4, space="PSUM") as pp,
        ):
            def load_w(oc):
                wc = wp.tile([P, IT, OC], bf16, tag="w", name="wc")
                for q in range(4):
                    nc.sync.dma_start(
                        out=wc[:, q * QI:(q + 1) * QI, :],
                        in_=w_d[oc, :, q * QI:(q + 1) * QI, :],
                    )
                return wc

            def load_x(sl):
                xs = xp.tile([P, IT, SLAB], bf16, tag="x", name="xs")
                for q in range(4):
                    nc.sync.dma_start(
                        out=xs[:, q * QI:(q + 1) * QI, :],
                        in_=x_d[sl, :, q * QI:(q + 1) * QI, :],
                    )
                return xs

            # PE warm-up: ~120 junk matmuls (no DMA deps, scheduled first)
            # keep the tensor engine busy through the HAM activity window
            # while the first real tiles are still in flight, so the real
            # matmuls start at the full 2.4 GHz clock.
            wj = xp.tile([P, P], bf16, tag="warm", name="wj")
            nc.vector.memset(wj[:], 0.0)
            psj = pp.tile([P, 64], f32, tag="psj", name="psj")
            for _ in range(120):
                nc.tensor.matmul(psj[:], wj[:], wj[:, :64], start=True, stop=True)

            for sl in range(NSL):
                if sl == 0:
                    # Interleave the first w chunk with the x slab in eighth
                    # chunks so the first accumulation can start ~1 MiB into
                    # the load.
                    wc0 = wp.tile([P, IT, OC], bf16, tag="w", name="wc")
                    xs = xp.tile([P, IT, SLAB], bf16, tag="x", name="xs")
                    E = IT // 8
                    for q in range(8):
                        nc.sync.dma_start(
                            out=wc0[:, q * E:(q + 1) * E, :],
                            in_=w_d[0, :, q * E:(q + 1) * E, :],
                        )
                        nc.sync.dma_start(
                            out=xs[:, q * E:(q + 1) * E, :],
                            in_=x_d[0, :, q * E:(q + 1) * E, :],
                        )
                else:
                    xs = load_x(sl)
                for oc in range(NOC):
                    wc = wc0 if sl == 0 and oc == 0 else load_w(oc)
                    bt = bp.tile([P, OC], f32, tag="b", name="bt")
                    nc.gpsimd.dma_start(out=bt[:], in_=b_d[oc])
                    for st in range(STS):
                        ps = pp.tile([P, OC], f32, tag="ps", name="ps")
                        for it in range(IT):
                            nc.tensor.matmul(
                                ps[:],
                                xs[:, it, st * P:(st + 1) * P],
                                wc[:, it, :],
                                start=(it == 0),
                                stop=(it == IT - 1),
                            )
                        ot = op.tile([P, OC], f32, tag="o", name="ot")
                        nc.vector.tensor_add(out=ot[:], in0=ps[:], in1=bt[:])
                        nc.scalar.dma_start(
                            out=o_d[
                                sl * SLAB + st * P:sl * SLAB + (st + 1) * P,
                                oc * OC:(oc + 1) * OC,
                            ],
                            in_=ot[:],
                        )
    nc.compile()
    return nc


def _install_axon_ntff_hook(so_path="/opt/axon/libaxon_pjrt.so"):
    """Make run_bass_kernel_spmd(trace=True) work when the image's antenv
    lacks axon_hooks: drive NTFF profiling via ctypes on libaxon_pjrt.so."""
    import contextlib
    import ctypes
    import sys
    import types

    lib = ctypes.CDLL(so_path)
    if not hasattr(lib, "axon_start_nrt_profile"):
        return
    lib.axon_start_nrt_profile.argtypes = [
        ctypes.POINTER(ctypes.c_int64),
        ctypes.c_size_t,
    ]
    lib.axon_start_nrt_profile.restype = ctypes.c_int64
    lib.axon_stop_nrt_profile.argtypes = [ctypes.c_char_p]
    lib.axon_stop_nrt_profile.restype = ctypes.c_int64

    @contextlib.contextmanager
    def _hook(output_dir, device_ids):
        import jax

        jax.devices()
        if device_ids:
            ids = (ctypes.c_int64 * len(device_ids))(*device_ids)
            rc = lib.axon_start_nrt_profile(ids, len(device_ids))
        else:
            rc = lib.axon_start_nrt_profile(None, 0)
        if rc != 0:
            raise RuntimeError(f"axon_start_nrt_profile rc={rc}")
        try:
            yield
        finally:
            n = lib.axon_stop_nrt_profile(str(output_dir).encode())
            print(f"ntff profile: {n} file(s) -> {output_dir}", file=sys.stderr)

    mod = types.ModuleType("antenv.axon_hooks")
    mod.get_axon_ntff_profile_hook = lambda: _hook
    mod.set_axon_ntff_profile_hook = lambda h: None
    sys.modules["antenv.axon_hooks"] = mod

    import concourse.bass_utils as bu

    bu.upload_artifacts = lambda tmpdir: f"file://{tmpdir}"


def kernel(x, weight, bias, block_mask):
    global LAST_EXEC_NS
    from concourse.bass_utils import run_bass_kernel_spmd
    from concourse import mybir

    bf16 = mybir.dt.np(mybir.dt.bfloat16)

    # Host-side prep: fold mask and the x2 into the weight, pre-transpose.
    mask = np.repeat(np.repeat(np.asarray(block_mask), BLOCK, 0), BLOCK, 1)
    w_eff = (2.0 * np.asarray(weight, np.float32)) * mask
    wt = np.ascontiguousarray(w_eff.T)                       # [IN, OUT]
    # [NOC, P, IT, OC]: per (oc, partition) a contiguous IT*OC*2-byte run.
    w_dev = np.ascontiguousarray(
        wt.reshape(IT, P, NOC, OC).transpose(2, 1, 0, 3)
    ).astype(bf16)
    b_dev = np.ascontiguousarray(
        np.broadcast_to(
            np.asarray(bias, np.float32).reshape(NOC, 1, OC), (NOC, P, OC)
        )
    )

    xs = np.asarray(x, np.float32)
    in_maps = []
    for b in range(B):
        # [NSL, P, IT, SLAB]: per (slab, partition) contiguous IT*SLAB*2 run.
        x_dev = np.ascontiguousarray(
            xs[b].T.reshape(IT, P, NSL, SLAB).transpose(2, 1, 0, 3)
        ).astype(bf16)
        in_maps.append({"xt": x_dev, "wt": w_dev, "bias": b_dev})

    nc = _build_program()
    trace = bool(int(os.environ.get("BSL_TRACE", "0")))
    if trace:
        _install_axon_ntff_hook()
    res = run_bass_kernel_spmd(
        nc, in_maps, list(range(B)), trace=trace,
    )
    LAST_EXEC_NS = res.exec_time_ns
    return np.stack([res.results[b]["out"] for b in range(B)]).astype(np.float32)


# revision 9
# speedup vs baseline: 1.0068x; 1.0009x over previous
"""Block-sparse linear kernel for Trainium2 (8 NeuronCores, data-parallel).

Computes out = 2 * (x @ (weight*mask).T) + bias for
x: (8, 2048, 4096) f32, weight: (4096, 4096) f32, bias: (4096,) f32,
block_mask: (128, 128) bool over 32x32 blocks.

Strategy: shard x on batch across the 8 cores (weight/bias replicated).
The mask and the *2 scale are folded into the weight on the host, so each
core runs a dense M=2048, K=4096, N=4096 GEMM in fp16 with fp32 PSUM
accumulation. Both operands stream: x in s-slabs of 512 rows, weight.T in
o-chunks of 512 (reloaded per slab -- HBM bandwidth has slack, and the
small working set lets compute start ~4 MiB into the transfer instead of
waiting for a full 16 MiB x residency). Transfers are batched into ~1 MiB
dma_starts to keep the Sync queue shallow; output stores issue from the
Scalar engine and bias loads from GpSimd so they never queue ahead of
weight loads. Bias is added during PSUM->SBUF eviction on the vector
engine.
"""
import os

import numpy as np

# Problem constants (hardcoded per the harness contract).
B, S, IN, OUT = 8, 2048, 4096, 4096
BLOCK = 32
P = 128                    # partitions / contraction tile
IT = IN // P               # 32 i-tiles
OC = 512                   # o-chunk width (matmul free dim)
NOC = OUT // OC            # 8 o-chunks
SLAB = 512                 # s rows per slab
NSL = S // SLAB            # 4 slabs
STS = SLAB // P            # 4 s-tiles per slab
QI = IT // 4               # i-tiles per DMA quarter

LAST_EXEC_NS = None


def _build_program():
    import concourse.bacc as bacc
    import concourse.tile as tile
    from concourse import mybir

    f16 = mybir.dt.float16
    f32 = mybir.dt.float32

    nc = bacc.Bacc("TRN2", debug=False, num_devices=B)
    x_d = nc.dram_tensor("xt", (NSL, P, IT, SLAB), f16, kind="ExternalInput")
    w_d = nc.dram_tensor("wt", (NOC, P, IT, OC), f16, kind="ExternalInput")
    b_d = nc.dram_tensor("bias", (NOC, P, OC), f32, kind="ExternalInput")
    o_d = nc.dram_tensor("out", (S, OUT), f32, kind="ExternalOutput")

    with tile.TileContext(nc) as tc:
        with (
            tc.tile_pool(name="xpool", bufs=2) as xp,
            tc.tile_pool(name="wpool", bufs=3) as wp,
            tc.tile_pool(name="bpool", bufs=2) as bp,
            tc.tile_pool(name="opool", bufs=4) as op,
            tc.tile_pool(name="psum", bufs=4, space="PSUM") as pp,
        ):
            def load_w(oc):
                wc = wp.tile([P, IT, OC], f16, tag="w", name="wc")
                for q in range(4):
                    nc.sync.dma_start(
                        out=wc[:, q * QI:(q + 1) * QI, :],
                        in_=w_d[oc, :, q * QI:(q + 1) * QI, :],
                    )
                return wc

            def load_x(sl):
                xs = xp.tile([P, IT, SLAB], f16, tag="x", name="xs")
                for q in range(4):
                    nc.sync.dma_start(
                        out=xs[:, q * QI:(q + 1) * QI, :],
                        in_=x_d[sl, :, q * QI:(q + 1) * QI, :],
                    )
                return xs

            # PE warm-up: ~120 junk matmuls (no DMA deps, scheduled first)
            # keep the tensor engine busy through the HAM activity window
            # while the first real tiles are still in flight, so the real
            # matmuls start at the full 2.4 GHz clock.
            wj = xp.tile([P, P], f16, tag="warm", name="wj")
            nc.vector.memset(wj[:], 0.0)
            psj = pp.tile([P, 64], f32, tag="psj", name="psj")
            for _ in range(120):
                nc.tensor.matmul(psj[:], wj[:], wj[:, :64], start=True, stop=True)

            for sl in range(NSL):
                if sl == 0:
                    # Interleave the first w chunk with the x slab in eighth
                    # chunks so the first accumulation can start ~1 MiB into
                    # the load.
                    wc0 = wp.tile([P, IT, OC], f16, tag="w", name="wc")
                    xs = xp.tile([P, IT, SLAB], f16, tag="x", name="xs")
                    E = IT // 8
                    for q in range(8):
                        nc.sync.dma_start(
                            out=wc0[:, q * E:(q + 1) * E, :],
                            in_=w_d[0, :, q * E:(q + 1) * E, :],
                        )
                        nc.sync.dma_start(
                            out=xs[:, q * E:(q + 1) * E, :],
                            in_=x_d[0, :, q * E:(q + 1) * E, :],
                        )
                else:
                    xs = load_x(sl)
                for oc in range(NOC):
                    wc = wc0 if sl == 0 and oc == 0 else load_w(oc)
                    bt = bp.tile([P, OC], f32, tag="b", name="bt")
                    nc.gpsimd.dma_start(out=bt[:], in_=b_d[oc])
                    for st in range(STS):
                        ps = pp.tile([P, OC], f32, tag="ps", name="ps")
                        for it in range(IT):
                            nc.tensor.matmul(
                                ps[:],
                                xs[:, it, st * P:(st + 1) * P],
                                wc[:, it, :],
                                start=(it == 0),
                                stop=(it == IT - 1),
                            )
                        ot = op.tile([P, OC], f32, tag="o", name="ot")
                        nc.vector.tensor_add(out=ot[:], in0=ps[:], in1=bt[:])
                        nc.scalar.dma_start(
                            out=o_d[
                                sl * SLAB + st * P:sl * SLAB + (st + 1) * P,
                                oc * OC:(oc + 1) * OC,
                            ],
                            in_=ot[:],
                        )
    nc.compile()
    return nc


def _install_axon_ntff_hook(so_path="/opt/axon/libaxon_pjrt.so"):
    """Make run_bass_kernel_spmd(trace=True) work when the image's antenv
    lacks axon_hooks: drive NTFF profiling via ctypes on libaxon_pjrt.so."""
    import contextlib
    import ctypes
    import sys
    import types

    lib = ctypes.CDLL(so_path)
    if not hasattr(lib, "axon_start_nrt_profile"):
        return
    lib.axon_start_nrt_profile.argtypes = [
        ctypes.POINTER(ctypes.c_int64),
        ctypes.c_size_t,
    ]
    lib.axon_start_nrt_profile.restype = ctypes.c_int64
    lib.axon_stop_nrt_profile.argtypes = [ctypes.c_char_p]
    lib.axon_stop_nrt_profile.restype = ctypes.c_int64

    @contextlib.contextmanager
    def _hook(output_dir, device_ids):
        import jax

        jax.devices()
        if device_ids:
            ids = (ctypes.c_int64 * len(device_ids))(*device_ids)
            rc = lib.axon_start_nrt_profile(ids, len(device_ids))
        else:
            rc = lib.axon_start_nrt_profile(None, 0)
        if rc != 0:
            raise RuntimeError(f"axon_start_nrt_profile rc={rc}")
        try:
            yield
        finally:
            n = lib.axon_stop_nrt_profile(str(output_dir).encode())
            print(f"ntff profile: {n} file(s) -> {output_dir}", file=sys.stderr)

    mod = types.ModuleType("antenv.axon_hooks")
    mod.get_axon_ntff_profile_hook = lambda: _hook
    mod.set_axon_ntff_profile_hook = lambda h: None
    sys.modules["antenv.axon_hooks"] = mod

    import concourse.bass_utils as bu

    bu.upload_artifacts = lambda tmpdir: f"file://{tmpdir}"


def kernel(x, weight, bias, block_mask):
    global LAST_EXEC_NS
    from concourse.bass_utils import run_bass_kernel_spmd
    from concourse import mybir

    f16 = np.float16

    # Host-side prep: fold mask and the x2 into the weight, pre-transpose.
    mask = np.repeat(np.repeat(np.asarray(block_mask), BLOCK, 0), BLOCK, 1)
    w_eff = (2.0 * np.asarray(weight, np.float32)) * mask
    wt = np.ascontiguousarray(w_eff.T)                       # [IN, OUT]
    # [NOC, P, IT, OC]: per (oc, partition) a contiguous IT*OC*2-byte run.
    w_dev = np.ascontiguousarray(
        wt.reshape(IT, P, NOC, OC).transpose(2, 1, 0, 3)
    ).astype(f16)
    b_dev = np.ascontiguousarray(
        np.broadcast_to(
            np.asarray(bias, np.float32).reshape(NOC, 1, OC), (NOC, P, OC)
        )
    )

    xs = np.asarray(x, np.float32)
    in_maps = []
    for b in range(B):
        # [NSL, P, IT, SLAB]: per (slab, partition) contiguous IT*SLAB*2 run.
        x_dev = np.ascontiguousarray(
            xs[b].T.reshape(IT, P, NSL, SLAB).transpose(2, 1, 0, 3)
        ).astype(f16)
        in_maps.append({"xt": x_dev, "wt": w_dev, "bias": b_dev})

    nc = _build_program()
    trace = bool(int(os.environ.get("BSL_TRACE", "0")))
    if trace:
        _install_axon_ntff_hook()
    res = run_bass_kernel_spmd(
        nc, in_maps, list(range(B)), trace=trace,
    )
    LAST_EXEC_NS = res.exec_time_ns
    return np.stack([res.results[b]["out"] for b in range(B)]).astype(np.float32)
